# revision 3
# baseline (speedup 1.0000x reference)
"""Trainium2 Bass kernel for nn_DataEmbedding (DataEmbedding: lagged-conv token
embedding + sinusoid positional + temporal embeddings).

Strategy (pure data parallel, batch sharded 2-per-core across 8 cores):
  out[b, t, :] = Lbig[b].T @ Abig  +  OneHot[b].T @ Tables  +  (pe + bias)

  - Lbig [126, S]: 18 time-shifted copies (shift d' = 0..17 <-> delta = d'-16)
    of the 7 channel rows of x[b], built on-device by 18 SBUF->SBUF DMAs from a
    host-padded channel-major copy of x. Row p = d'*7 + c.
    The circular-conv + lag-mask edge cases collapse into: (a) 6 wrap values
    planted by the host in the left pad, (b) a [126,16] left-edge mask, (c) a
    [126,1] right-edge mask.
  - Abig [126, 512]: block-diagonal repack of the two conv kernels (host prep).
  - OneHot [28, S] bf16: built on-device with tensor_scalar(is_equal) from
    broadcast x_mark rows; row p = e*4 + m. Tables split hi/lo bf16 so the
    temporal matmuls run at full PE rate with ~2^-17 error.
  - pe + conv bias folded into one [S, 512] table (host prep), added on DVE.
"""

import math

import numpy as np
import ml_dtypes

import concourse.bass as bass
import concourse.mybir as mybir
import concourse.tile as tile
from concourse import bacc
from concourse.bass_utils import run_bass_kernel_spmd

# problem constants (hardcoded per harness contract)
B, S, CIN = 16, 4096, 7
TAO, M, D = 3, 5, 512
KER = 73  # D // CIN
NSH = 18  # shifts d' = 0..17, delta = d' - 16
K_CONV = CIN * NSH  # 126
XOFF = 16
XW = 4128  # 16 left pad + 4096 + right pad
N_CORES = 8
B_PER = B // N_CORES  # 2
N_TILES = S // 128  # 32
F32 = mybir.dt.float32
BF16 = mybir.dt.bfloat16
BF16_NP = ml_dtypes.bfloat16


def _sinusoid_table(n, d):
    """float64 sinusoid table rounded to f32 (matches jax's f32 table to ~1ulp
    of jax's own rounding error)."""
    pos = np.arange(n, dtype=np.float64)[:, None]
    div = np.exp(np.arange(0, d, 2, dtype=np.float64) * (-math.log(10000.0) / d))
    tab = np.zeros((n, d), np.float64)
    tab[:, 0::2] = np.sin(pos * div)
    tab[:, 1::2] = np.cos(pos * div)
    return tab.astype(np.float32)


def host_prep(x, x_mark, conv_w, conv_b, left_w, left_b):
    """All the data-layout shuffling the device shouldn't waste DMA on."""
    x = np.ascontiguousarray(np.asarray(x, np.float32))
    conv_w = np.asarray(conv_w, np.float32)
    conv_b = np.asarray(conv_b, np.float32)
    left_w = np.asarray(left_w, np.float32)
    left_b = np.asarray(left_b, np.float32)

    # channel-major padded x: xt[b, c, XOFF+s] = x[b, s, c]
    xt = np.zeros((B, CIN, XW), np.float32)
    xt[:, :, XOFF:XOFF + S] = x.transpose(0, 2, 1)
    # t=0 wrap values for the j=0 shift blocks: xt[c, u] = x[b, 4080+u, c]
    for u in range(0, XOFF, TAO):
        xt[:, :, u] = x[:, S - XOFF + u, :]

    # marks, channel-major bf16: [B, 4, S]
    marks = np.ascontiguousarray(
        np.asarray(x_mark)[:, :, :4].transpose(0, 2, 1)).astype(BF16_NP)

    # Abig [126, 512], row p = d'*7 + c
    abig = np.zeros((K_CONV, D), np.float32)
    for i in range(M + 1):
        for j in range(3):
            dp = 15 + j - 3 * i
            for c in range(CIN):
                abig[dp * 7 + c, c * KER:(c + 1) * KER] = conv_w[:, i, j]
            abig[dp * 7 + 6, D - 1] += left_w[0, i, j]

    # left-edge mask [126, 16] and right-edge mask [126, 1]
    mask16 = np.zeros((K_CONV, 16), np.float32)
    maskr = np.ones((K_CONV, 1), np.float32)
    for dp in range(NSH):
        thr = 16 - (dp % 3)
        mask16[dp * 7:(dp + 1) * 7, thr:] = 1.0
        if dp % 3 == 0:
            mask16[dp * 7:(dp + 1) * 7, 0] = 1.0
        if dp % 3 == 2:
            maskr[dp * 7:(dp + 1) * 7, 0] = 0.0

    # pe + conv bias table [S, D]
    bias = np.zeros(D, np.float32)
    for c in range(CIN):
        bias[c * KER:(c + 1) * KER] = conv_b
    bias[D - 1] = left_b[0]
    pe_bias = _sinusoid_table(S, D) + bias[None, :]

    # temporal tables [28, 512] hi/lo bf16, row p = e*4 + m
    sizes = [13, 32, 7, 24]
    tabs = np.zeros((28, D), np.float32)
    for m in range(4):
        t = _sinusoid_table(sizes[m], D)
        for e in range(7):
            tabs[e * 4 + m] = t[e]
    tabs_hi = tabs.astype(BF16_NP)
    tabs_lo = (tabs - tabs_hi.astype(np.float32)).astype(BF16_NP)

    abig_hi = abig.astype(BF16_NP)
    abig_lo = (abig - abig_hi.astype(np.float32)).astype(BF16_NP)

    evals = np.ascontiguousarray((np.arange(28, dtype=np.float32) // 4).reshape(28, 1))
    return (xt, marks, abig_hi, abig_lo, mask16, maskr, pe_bias,
            tabs_hi, tabs_lo, evals)


def build_nc(reps=1, skip=()):
    """Build the per-core Bass program (B_PER batches per core)."""
    nc = bacc.Bacc("TRN2", target_bir_lowering=False, debug=False)

    xt_d = nc.dram_tensor("xt", [B_PER, CIN, XW], F32, kind="ExternalInput").ap()
    marks_d = nc.dram_tensor("marks", [B_PER, 4, S], BF16, kind="ExternalInput").ap()
    abhi_d = nc.dram_tensor("abig_hi", [K_CONV, D], BF16, kind="ExternalInput").ap()
    ablo_d = nc.dram_tensor("abig_lo", [K_CONV, D], BF16, kind="ExternalInput").ap()
    mask16_d = nc.dram_tensor("mask16", [K_CONV, 16], F32, kind="ExternalInput").ap()
    maskr_d = nc.dram_tensor("maskr", [K_CONV, 1], F32, kind="ExternalInput").ap()
    pe_d = nc.dram_tensor("pe_bias", [S, D], F32, kind="ExternalInput").ap()
    tabhi_d = nc.dram_tensor("tabs_hi", [28, D], BF16, kind="ExternalInput").ap()
    tablo_d = nc.dram_tensor("tabs_lo", [28, D], BF16, kind="ExternalInput").ap()
    evals_d = nc.dram_tensor("evals", [28, 1], F32, kind="ExternalInput").ap()
    out_d = nc.dram_tensor("out", [B_PER, S, D], F32, kind="ExternalOutput").ap()

    # [B_PER, n, 128, 512] views for per-t-tile DMA
    out_v = out_d.rearrange("b (n p) d -> b n p d", p=128)
    pe_v = pe_d.rearrange("(n p) d -> n p d", p=128)

    with tile.TileContext(nc) as tc:
        with (
            tc.tile_pool(name="consts", bufs=1) as consts,
            tc.tile_pool(name="persist", bufs=1) as persist,
            tc.tile_pool(name="stream", bufs=4) as stream,
            tc.tile_pool(name="psum", bufs=4, space="PSUM") as psum_pool,
        ):
            abhi_sb = consts.tile([K_CONV, D], BF16, tag="abhi")
            nc.sync.dma_start(abhi_sb[:], abhi_d[:])
            ablo_sb = consts.tile([K_CONV, D], BF16, tag="ablo")
            nc.sync.dma_start(ablo_sb[:], ablo_d[:])
            tabhi_sb = consts.tile([28, D], BF16, tag="tabhi")
            nc.sync.dma_start(tabhi_sb[:], tabhi_d[:])
            tablo_sb = consts.tile([28, D], BF16, tag="tablo")
            nc.sync.dma_start(tablo_sb[:], tablo_d[:])
            evals_sb = consts.tile([28, 1], F32, tag="evals")
            nc.sync.dma_start(evals_sb[:], evals_d[:])
            mask16_sb = consts.tile([K_CONV, 16], F32, tag="mask16")
            nc.sync.dma_start(mask16_sb[:], mask16_d[:])
            maskr_sb = consts.tile([K_CONV, 1], F32, tag="maskr")
            nc.sync.dma_start(maskr_sb[:], maskr_d[:])

            def body(_iv=None):
                lbig = []
                oh = []
                for b in range(B_PER):
                    xt_sb = persist.tile([CIN, XW], F32, tag=f"xt{b}")
                    nc.sync.dma_start(xt_sb[:], xt_d[b])

                    lb = persist.tile([K_CONV, S], F32, tag=f"lbig{b}")
                    if "shift" not in skip:
                        for dp in range(NSH):
                            nc.sync.dma_start(
                                lb[dp * 7:(dp + 1) * 7, :], xt_sb[:, dp:dp + S])
                    # edge masks
                    nc.vector.tensor_mul(lb[:, 0:16], lb[:, 0:16], mask16_sb[:])
                    nc.vector.tensor_mul(
                        lb[:, S - 1:S], lb[:, S - 1:S], maskr_sb[:])
                    lb_hi = persist.tile([K_CONV, S], BF16, tag=f"lbhi{b}")
                    nc.vector.tensor_copy(lb_hi[:], lb[:])
                    lb_lo = persist.tile([K_CONV, S], BF16, tag=f"lblo{b}")
                    nc.vector.tensor_sub(lb_lo[:], lb[:], lb_hi[:])
                    lbig.append((lb_hi, lb_lo))

                    o = persist.tile([28, S], BF16, tag=f"oh{b}")
                    if "oh" not in skip:
                        for e in range(7):
                            nc.sync.dma_start(o[e * 4:(e + 1) * 4, :], marks_d[b])
                    nc.vector.tensor_scalar(
                        out=o[:], in0=o[:], scalar1=evals_sb[:], scalar2=None,
                        op0=mybir.AluOpType.is_equal)
                    oh.append(o)

                for ti in range(N_TILES):
                    pe_sb = stream.tile([128, D], F32, tag="pe")
                    if "pe" not in skip:
                        nc.sync.dma_start(pe_sb[:], pe_v[ti])
                    for b in range(B_PER):
                        ts = slice(ti * 128, (ti + 1) * 128)
                        lb_hi, lb_lo = lbig[b]
                        out_sb = stream.tile([128, D], F32, tag="out")
                        if "mm" not in skip:
                            ps = psum_pool.tile([128, D], F32, tag="ps")
                            nc.tensor.matmul(ps[:], lb_hi[:, ts], abhi_sb[:],
                                             start=True, stop=False)
                            nc.tensor.matmul(ps[:], lb_hi[:, ts], ablo_sb[:],
                                             start=False, stop=False)
                            nc.tensor.matmul(ps[:], lb_lo[:, ts], abhi_sb[:],
                                             start=False, stop=False)
                            nc.tensor.matmul(ps[:], oh[b][:, ts], tabhi_sb[:],
                                             start=False, stop=False)
                            nc.tensor.matmul(ps[:], oh[b][:, ts], tablo_sb[:],
                                             start=False, stop=True)
                            nc.vector.tensor_add(out_sb[:], ps[:], pe_sb[:])
                        else:
                            nc.vector.tensor_copy(out_sb[:], pe_sb[:])
                        if "out" not in skip:
                            nc.scalar.dma_start(out_v[b, ti], out_sb[:])

            if reps == 1:
                body()
            else:
                with tc.For_i(0, reps, 1) as iv:
                    body(iv)
    nc.compile()
    return nc


_NC_CACHE = {}


def _get_nc(reps=1):
    if reps not in _NC_CACHE:
        _NC_CACHE[reps] = build_nc(reps)
    return _NC_CACHE[reps]


def kernel(x, x_mark, conv_w, conv_b, left_w, left_b, _reps=1, _return_results=False,
           _trace=False, _tmpdir=None):
    (xt, marks, abig_hi, abig_lo, mask16, maskr, pe_bias,
     tabs_hi, tabs_lo, evals) = host_prep(
        x, x_mark, conv_w, conv_b, left_w, left_b)

    in_maps = []
    for core in range(N_CORES):
        bs = slice(core * B_PER, (core + 1) * B_PER)
        in_maps.append({
            "xt": np.ascontiguousarray(xt[bs]),
            "marks": np.ascontiguousarray(marks[bs]),
            "abig_hi": abig_hi,
            "abig_lo": abig_lo,
            "mask16": mask16,
            "maskr": maskr,
            "pe_bias": pe_bias,
            "tabs_hi": tabs_hi,
            "tabs_lo": tabs_lo,
            "evals": evals,
        })

    nc = _get_nc(_reps)
    kw = {}
    if _trace:
        kw = dict(trace=True, tmpdir=_tmpdir)
    res = run_bass_kernel_spmd(nc, in_maps, core_ids=list(range(N_CORES)), **kw)
    out = np.concatenate([r["out"] for r in res.results], axis=0)
    assert out.shape == (B, S, D)
    if _return_results:
        return out, res
    return out

